# revision 20
# baseline (speedup 1.0000x reference)
"""Causal self-attention kernel for 8 Trainium2 NeuronCores.

Sharding: core c -> (batch b = c//2, head-group g = c%2). Each core computes
the attention output contribution of 8 heads for one batch element:
    P_c = (sum_{h in group} softmax(Q_h K_h^T / 8 + causal) V_h) @ WO
Host epilogue: out[b] = P_{2b} + P_{2b+1} + (sum_h bV_h) @ WO + 16*bO
(the V-bias commutes through softmax normalization: softmax rows sum to 1).

All matmul operands are fp16 (1 cycle/row on the PE; every tensor here is
O(1) so fp16's 10 mantissa bits beat bf16 and cost half of fp32r's
LOW_HIGH two-pass mode). Accumulation stays fp32 in PSUM; softmax
normalization is all fp32.

Per core:
  xT[d, s]        x cast to fp16 on host, DMA-transposed on load
  QT[dh, s], KT   = W.T @ xT, bias added via a K=1 rank-1 matmul (b x ones)
  V_aug[s, 65/hd] = x @ WV plus a ones column per head
  ST[k, q]        two heads packed per PE pass via tile_position (0,0)/(64,0)
                  into one [128,1024] PSUM tile
  causal          additive -1e6 triangular mask on the diagonal 128-block in
                  PSUM; fully-masked column blocks skip exp (strided-AP
                  ACTIVATE) and are zero-filled by gpsimd memset
  ET fp16         = exp(ST/8) on ScalarE; ZT[65,q] += V_aug.T @ ET in PSUM
                  (ZT is software-pipelined one k-step behind ST)
  l -> 1/l        ZT row 64; per q-chunk batch: in-place Ln then Exp(-x) on
                  ScalarE (same ACT table set as the main exp), broadcast
                  across partitions by a partition-step-0 DMA
  out             = (sum_h ZT_h * (1/l_h)).T @ WO
"""
import numpy as np

B, S, D, H, DH = 4, 2048, 1024, 16, 64
HPC = 8            # heads per core
GD = HPC * DH      # 512 = group width
NCORES = 8
NQ = S // 512      # 4 q/s chunks of 512
NKT = S // 128     # 16 k-tiles
NDT = D // 128     # 8 d-tiles

_prog = {}


def bass_ap_3d(tile_t, offset, stride, n, inner):
    """AP view [128p, n, inner] over a tile's free dim: col = offset + i*stride + c."""
    import concourse.bass as bass
    ap = tile_t[:]
    return bass.AP(ap.tensor, ap.offset + offset,
                   [ap.ap[0], [stride, n], [1, inner]])


def _bcast_ap(tile_t, row, col, nparts, width):
    """Partition-step-0 AP reading (row, col:col+width) replicated nparts times."""
    import concourse.bass as bass
    ap = tile_t[:]
    pstep = ap.ap[0][0]
    return bass.AP(ap.tensor, ap.offset + row * pstep + col,
                   [[0, nparts], [1, width]])


def _build():
    import concourse.bacc as bacc
    import concourse.tile as tile
    from concourse import mybir

    f32 = mybir.dt.float32
    f16 = mybir.dt.float16
    AF = mybir.ActivationFunctionType
    ALU = mybir.AluOpType

    nc = bacc.Bacc(None, target_bir_lowering=False, debug=False)
    x = nc.dram_tensor("x", [S, D], f16, kind="ExternalInput")
    wq = nc.dram_tensor("wq", [D, GD], f16, kind="ExternalInput")
    wk = nc.dram_tensor("wk", [D, GD], f16, kind="ExternalInput")
    wv = nc.dram_tensor("wv", [D, GD], f16, kind="ExternalInput")
    bq = nc.dram_tensor("bq", [1, GD], f16, kind="ExternalInput")
    bk = nc.dram_tensor("bk", [1, GD], f16, kind="ExternalInput")
    wo = nc.dram_tensor("wo", [DH, D], f16, kind="ExternalInput")
    out = nc.dram_tensor("out", [S, D], f32, kind="ExternalOutput")

    with tile.TileContext(nc) as tc:
        with tc.tile_pool(name="const", bufs=1) as constp, \
             tc.tile_pool(name="big", bufs=1) as bigp:
            # constants
            ones64 = constp.tile([128, DH], f16, tag="ones64")
            nc.vector.memset(ones64[64:65, :], 1.0)
            idt = constp.tile([128, 128], f16, tag="idt")
            from concourse.masks import make_identity
            make_identity(nc, idt[:])
            import concourse.bass as bass
            bq_t = constp.tile([128, 4], f32, tag="bq_t")
            bk_t = constp.tile([128, 4], f32, tag="bk_t")
            nc.gpsimd.dma_start(bq_t[:], bass.AP(bq, 0, [[1, 128], [128, 4]]))
            nc.gpsimd.dma_start(bk_t[:], bass.AP(bk, 0, [[1, 128], [128, 4]]))
            wo_sb = constp.tile([DH, D], f16, tag="wo_sb")
            nc.sync.dma_start(wo_sb[:], wo[:])

            # persistent per-core tensors
            xt_all = bigp.tile([128, NDT * S], f16, tag="xt")  # d-tile j at cols j*S
            qt_all = bigp.tile([128, 4 * S], f16, tag="qt")    # m-tile m at cols m*S
            kt_all = bigp.tile([128, 4 * S], f16, tag="kt")
            vt_all = bigp.tile([128, NKT * 520], f16, tag="vt")
            zsum = bigp.tile([DH, S], f32, tag="zsum")

            # -------- streamed: per 512-chunk proj -> attention -> tail --------
            # one unified 2-bank-slot PSUM pool (tag "st2") serves projection
            # accumulators, transpose staging, score tiles and out-proj
            with tc.tile_pool(name="wts", bufs=1) as wtp, \
                 tc.tile_pool(name="xs", bufs=3) as xsp, \
                 tc.tile_pool(name="et", bufs=8) as etp, \
                 tc.tile_pool(name="ztall", bufs=2) as zta_p, \
                 tc.tile_pool(name="rld", bufs=2, space="DRAM") as rldp, \
                 tc.tile_pool(name="lbs", bufs=1) as lbsp, \
                 tc.tile_pool(name="zn", bufs=2) as znp, \
                 tc.tile_pool(name="zr", bufs=2) as zrp, \
                 tc.tile_pool(name="osb", bufs=3) as osbp, \
                 tc.tile_pool(name="stp", bufs=3, space="PSUM") as stp, \
                 tc.tile_pool(name="ztp", bufs=2, space="PSUM") as ztp:
                wq_sb = [wtp.tile([128, GD], f16, tag=f"wq{k}", name=f"wq{k}")
                         for k in range(NDT)]
                wk_sb = [wtp.tile([128, GD], f16, tag=f"wk{k}", name=f"wk{k}")
                         for k in range(NDT)]
                wv_sb = [wtp.tile([128, GD], f16, tag=f"wv{k}", name=f"wv{k}")
                         for k in range(NDT)]
                for k in range(NDT):
                    nc.gpsimd.dma_start(wq_sb[k][:], wq[k * 128:(k + 1) * 128, :])
                    nc.gpsimd.dma_start(wk_sb[k][:], wk[k * 128:(k + 1) * 128, :])
                    nc.gpsimd.dma_start(wv_sb[k][:], wv[k * 128:(k + 1) * 128, :])

                def proj(nq):
                    # transpose this 512-row slice of x into xT via the PE
                    for st4 in range(4):
                        srow = nq * 512 + st4 * 128
                        xs = xsp.tile([128, D], f16, tag="xs", name="xs")
                        nc.sync.dma_start(xs[:], x[srow:srow + 128, :])
                        for jj in range(2):
                            pt = stp.tile([128, 1024], f16, tag="st2", name="pt")
                            for j4 in range(4):
                                j = jj * 4 + j4
                                nc.tensor.transpose(
                                    pt[:, j4 * 128:(j4 + 1) * 128],
                                    xs[:, j * 128:(j + 1) * 128], idt[:])
                            dst = bass_ap_3d(xt_all, (jj * 4) * S + srow, S, 4, 128)
                            srcap = bass_ap_3d(pt, 0, 128, 4, 128)
                            nc.vector.tensor_copy(dst, srcap)
                    # QT / KT for this chunk
                    for (w_sb, b_t, dest) in ((wq_sb, bq_t, qt_all),
                                              (wk_sb, bk_t, kt_all)):
                        for m in range(4):
                            ps = stp.tile([128, 1024], f32, tag="st2", name="ps")
                            for k in range(NDT):
                                nc.tensor.matmul(
                                    ps[:, 0:512], w_sb[k][:, m * 128:(m + 1) * 128],
                                    xt_all[:, k * S + nq * 512: k * S + (nq + 1) * 512],
                                    start=(k == 0), stop=(k == NDT - 1))
                            nc.vector.tensor_scalar_add(
                                dest[:, m * S + nq * 512: m * S + (nq + 1) * 512],
                                ps[:, 0:512], b_t[:, m:m + 1])
                    # V for this chunk
                    for m in range(4):
                        st = nq * 4 + m
                        ps = stp.tile([128, 1024], f32, tag="st2", name="ps")
                        for k in range(NDT):
                            nc.tensor.matmul(
                                ps[:, 0:512],
                                xt_all[:, k * S + st * 128: k * S + (st + 1) * 128],
                                wv_sb[k][:], start=(k == 0), stop=(k == NDT - 1))
                        dst = bass_ap_3d(vt_all, st * 520, 65, HPC, DH)
                        srcap = bass_ap_3d(ps, 0, DH, HPC, DH)
                        nc.vector.tensor_copy(dst, srcap)
                        nc.vector.memset(
                            bass_ap_3d(vt_all, st * 520 + DH, 65, HPC, 1), 1.0)

                ztalls = {}

                def attention(qc, hooks=None):
                    hooks = hooks or {}
                    ktiles = 4 * qc + 4
                    ztall = zta_p.tile([65, HPC * 512], f32, tag="ztall",
                                       name=f"ztall{qc}")
                    ztalls[qc] = ztall
                    for hp in range(4):
                        zt0 = ztp.tile([65, 512], f32, tag="zt", name="zt0")
                        zt1 = ztp.tile([65, 512], f32, tag="zt", name="zt1")
                        pending = []  # software pipeline: ZT lags ST by two k

                        def flush_zt(lag):
                            while len(pending) > lag:
                                pkt, pet = pending.pop(0)
                                nc.tensor.matmul(
                                    zt0[:], vt_all[:, pkt * 520 + (2 * hp) * 65:
                                                   pkt * 520 + (2 * hp) * 65 + 65],
                                    pet[:, 0:512],
                                    start=(pkt == 0), stop=(pkt == ktiles - 1))
                                nc.tensor.matmul(
                                    zt1[:], vt_all[:, pkt * 520 + (2 * hp + 1) * 65:
                                                   pkt * 520 + (2 * hp + 1) * 65 + 65],
                                    pet[:, 512:1024],
                                    start=(pkt == 0), stop=(pkt == ktiles - 1))
                        for kt in range(ktiles):
                            st2 = stp.tile([128, 1024], f32, tag="st2", name="st2")
                            nc.tensor.matmul(
                                st2[:, 0:512],
                                kt_all[0:64, hp * S + kt * 128: hp * S + (kt + 1) * 128],
                                qt_all[0:64, hp * S + qc * 512: hp * S + (qc + 1) * 512],
                                start=True, stop=True, tile_position=(0, 0))
                            nc.tensor.matmul(
                                st2[:, 512:1024],
                                kt_all[64:128, hp * S + kt * 128: hp * S + (kt + 1) * 128],
                                qt_all[64:128, hp * S + qc * 512: hp * S + (qc + 1) * 512],
                                start=True, stop=True, tile_position=(64, 0))
                            j = kt - 4 * qc
                            et = etp.tile([128, 1024], f16, tag="et", name="et")
                            if j > 0:
                                # left-of-diagonal q-subtiles fully masked:
                                # skip their exp, zero-fill ET
                                nc.gpsimd.memset(
                                    bass_ap_3d(et, 0, 512, 2, j * 128), 0.0)
                                nc.scalar.activation(
                                    bass_ap_3d(et, j * 128, 512, 2, 512 - j * 128),
                                    bass_ap_3d(st2, j * 128, 512, 2, 512 - j * 128),
                                    AF.Exp, scale=0.125)
                            else:
                                nc.scalar.activation(et[:], st2[:], AF.Exp,
                                                     scale=0.125)
                            if j >= 0:
                                # causal mask on the diagonal 128-block of ET
                                for half in range(2):
                                    blk = et[:, half * 512 + j * 128:
                                             half * 512 + (j + 1) * 128]
                                    nc.gpsimd.affine_select(
                                        out=blk, in_=blk, compare_op=ALU.is_ge,
                                        fill=0.0, base=0, pattern=[[1, 128]],
                                        channel_multiplier=-1)
                            pending.append((kt, et))
                            flush_zt(4)
                        flush_zt(0)
                        nc.scalar.activation(
                            ztall[:, (2 * hp) * 512:(2 * hp + 1) * 512], zt0[:],
                            AF.Copy)
                        nc.scalar.activation(
                            ztall[:, (2 * hp + 1) * 512:(2 * hp + 2) * 512], zt1[:],
                            AF.Copy)
                        if hp in hooks:
                            hooks[hp]()

                def tail_norm(qc, half, ztall):
                    cols = slice(half * 2048, (half + 1) * 2048)
                    nc.scalar.activation(ztall[64:65, cols], ztall[64:65, cols],
                                         AF.Ln)
                    nc.scalar.activation(ztall[64:65, cols], ztall[64:65, cols],
                                         AF.Exp, scale=-1.0)
                    rld = rldp.tile([1, 2048], f32, tag="rld")
                    nc.sync.dma_start(rld[:], ztall[64:65, cols])
                    lbs_all = lbsp.tile([DH, 2048], f32, tag="lbs")
                    nc.sync.dma_start(lbs_all[:], _bcast_ap(rld, 0, 0, DH, 2048))
                    for h4 in range(4):
                        hh = half * 4 + h4
                        lbs = lbs_all[:, h4 * 512:(h4 + 1) * 512]
                        if hh == 0:
                            nc.vector.tensor_tensor(
                                zsum[:, qc * 512:(qc + 1) * 512],
                                ztall[0:DH, hh * 512:(hh + 1) * 512],
                                lbs, op=ALU.mult)
                        else:
                            zn = znp.tile([DH, 512], f32, tag="zn")
                            nc.vector.tensor_tensor(
                                zn[:], ztall[0:DH, hh * 512:(hh + 1) * 512],
                                lbs, op=ALU.mult)
                            nc.vector.tensor_tensor(
                                zsum[:, qc * 512:(qc + 1) * 512],
                                zsum[:, qc * 512:(qc + 1) * 512],
                                zn[:], op=ALU.add)

                def tail_proj(qc):
                    zsr = zrp.tile([DH, 512], f16, tag="zsr")
                    nc.vector.tensor_copy(zsr[:], zsum[:, qc * 512:(qc + 1) * 512])
                    for qt in range(4):
                        for nn in range(2):
                            po = stp.tile([128, 1024], f32, tag="st2", name="po")
                            nc.tensor.matmul(
                                po[:, 0:512], zsr[:, qt * 128:(qt + 1) * 128],
                                wo_sb[:, nn * 512:(nn + 1) * 512],
                                start=True, stop=True)
                            osb = osbp.tile([128, 512], f32, tag="osb")
                            nc.vector.tensor_copy(osb[:], po[:, 0:512])
                            nc.sync.dma_start(
                                out[qc * 512 + qt * 128: qc * 512 + (qt + 1) * 128,
                                    nn * 512:(nn + 1) * 512], osb[:])

                def tail(qc):
                    ztall = ztalls.pop(qc)
                    cols = slice(0, HPC * 512)
                    nc.scalar.activation(ztall[64:65, cols], ztall[64:65, cols],
                                         AF.Ln)
                    nc.scalar.activation(ztall[64:65, cols], ztall[64:65, cols],
                                         AF.Exp, scale=-1.0)
                    rld = rldp.tile([1, HPC * 512], f32, tag="rld")
                    nc.sync.dma_start(rld[:], ztall[64:65, cols])
                    lbs_all = lbsp.tile([DH, HPC * 512], f32, tag="lbs")
                    nc.sync.dma_start(lbs_all[:], _bcast_ap(rld, 0, 0, DH, HPC * 512))
                    for hh in range(HPC):
                        lbs = lbs_all[:, hh * 512:(hh + 1) * 512]
                        if hh == 0:
                            nc.vector.tensor_tensor(
                                zsum[:, qc * 512:(qc + 1) * 512],
                                ztall[0:DH, hh * 512:(hh + 1) * 512],
                                lbs, op=ALU.mult)
                        else:
                            zn = znp.tile([DH, 512], f32, tag="zn")
                            nc.vector.tensor_tensor(
                                zn[:], ztall[0:DH, hh * 512:(hh + 1) * 512],
                                lbs, op=ALU.mult)
                            nc.vector.tensor_tensor(
                                zsum[:, qc * 512:(qc + 1) * 512],
                                zsum[:, qc * 512:(qc + 1) * 512],
                                zn[:], op=ALU.add)
                    tail_proj(qc)

                for nq in range(NQ):
                    proj(nq)
                    attention(nq)
                    if nq > 0:
                        tail(nq - 1)
                tail(NQ - 1)
    nc.compile()
    return nc


def kernel(**inputs):
    x = np.asarray(inputs["x"], dtype=np.float32)
    WQ = np.asarray(inputs["WQ"], dtype=np.float32)
    bQ = np.asarray(inputs["bQ"], dtype=np.float32)
    WK = np.asarray(inputs["WK"], dtype=np.float32)
    bK = np.asarray(inputs["bK"], dtype=np.float32)
    WV = np.asarray(inputs["WV"], dtype=np.float32)
    bV = np.asarray(inputs["bV"], dtype=np.float32)
    WO = np.asarray(inputs["WO"], dtype=np.float32)
    bO = np.asarray(inputs["bO"], dtype=np.float32)

    from concourse.bass_utils import run_bass_kernel_spmd

    if "nc" not in _prog:
        _prog["nc"] = _build()
    nc = _prog["nc"]

    in_maps = []
    for c in range(NCORES):
        b, g = c // 2, c % 2
        sl = slice(g * GD, (g + 1) * GD)
        in_maps.append({
            "x": np.ascontiguousarray(x[b]).astype(np.float16),
            "wq": np.ascontiguousarray(WQ[:, sl]).astype(np.float16),
            "wk": np.ascontiguousarray(WK[:, sl]).astype(np.float16),
            "wv": np.ascontiguousarray(WV[:, sl]).astype(np.float16),
            "bq": np.ascontiguousarray(bQ[sl]).reshape(1, GD).astype(np.float16),
            "bk": np.ascontiguousarray(bK[sl]).reshape(1, GD).astype(np.float16),
            "wo": WO.astype(np.float16),
        })
    _prog["in_maps"] = in_maps
    res = run_bass_kernel_spmd(nc, in_maps, core_ids=list(range(NCORES)))
    parts = [r["out"] for r in res.results]

    extra = bV.reshape(H, DH).sum(0) @ WO + np.float32(H) * bO
    out = np.empty((B, S, D), dtype=np.float32)
    for b in range(B):
        out[b] = parts[2 * b] + parts[2 * b + 1] + extra
    return out
